# revision 42
# baseline (speedup 1.0000x reference)
"""Trainium2 Bass kernel for nn_AttentionAggregator3d.

Math (per batch b):
    zmf = zm.reshape(CM, N)                     # N = D*W*H = 4096 tokens
    q = Wq @ zmf + bq ; k = Wk @ zmf + bk       # (16, N)
    v = Wv @ zmf + bv                           # (128, N)
    A = softmax_n(q^T k)                        # (N, N), softmax over keys n
    out = v @ A^T ; result = zc + gamma * out

Kernel structure (v3):
  * logits^T[n,m] = zm_n^T (Wk^T Wq) zm_m.  The kernel computes
    tq = (Wq^T Wk) @ zm[:, queries] once (the query block is only 1024
    wide) and then logits chunk j = zm_chunk_j^T @ tq — the stationary
    operand comes straight from zm in SBUF, so there is no per-chunk
    PSUM->SBUF staging of a projected tensor.
  * bq/bk enter softmax only through the per-key term
    r[n] = (Wk^T bq)·zm[:,n] (per-query terms cancel).
  * Sharding: 8 cores = batch (2) x query-block (4, 1024 queries each);
    zm rotated per core so its query block is at columns 0:1024.
  * E = exp(logits^T) kept keys-on-partitions in bf16 (half SBUF, PV in
    bf16 at full PE rate).  Softmax denominators are split three ways by
    measured engine cost: PE ones-matmuls into a PSUM accumulator (8
    chunks, exact f32), gpsimd bf16 adds (8), DVE bf16 adds over two
    accumulators (16).  gamma folds into Wv^T and gamma*bv into zc on
    the host, so the epilogue is reciprocal -> broadcast-matmul ->
    multiply -> add.
  * Input DMAs are issued in parallel across the three DMA-capable
    sequencers; weights / ones arrive as two packed DMAs (ones can't be
    memset in f32r/bf16 — the BIR verifier tracks dtype producers).
"""

import os
import sys
import types

import ml_dtypes
import numpy as np

import concourse.bacc as bacc_mod
import concourse.tile as tile
from concourse import mybir
from concourse.bass_utils import run_bass_kernel_spmd

B, CC, CM, P = 2, 128, 128, 16
N = 16 * 16 * 16          # 4096 tokens
MBLK = N // 4             # 1024 queries per core
NCORES = 8
NCHUNK = N // 128         # 32 key chunks of 128

F32 = mybir.dt.float32
F32R = mybir.dt.float32r
BF16 = mybir.dt.bfloat16
AF = mybir.ActivationFunctionType
ALU = mybir.AluOpType

LAST_RESULTS = None  # BassKernelResults of the most recent run (for test.py)

# packed bf16 weight layout (gt ships separately as f32r — the BIR
# verifier requires f32r matmul inputs to be produced as f32r)
WP_WVT = slice(0, 128)       # wvt bf16 (128 cols)
WP_ONE = slice(128, 129)     # ones bf16 (1 col; col 129 pads to 4B)
WP_COLS = 130
GT_COLS = 256                # gtT f32r (128 cols) + ones f32r (128 cols)


def _ensure_ntff_hook() -> bool:
    """The grading image lacks antenv.axon_hooks; synthesize it from the
    boot module's ctypes NTFF driver so trace=True works under axon."""
    try:
        import antenv.axon_hooks  # noqa: F401

        return True
    except ImportError:
        pass
    try:
        import antenv
        from trn_agent_boot.trn_boot import _ntff_profile_via_ctypes

        hook = _ntff_profile_via_ctypes("/opt/axon/libaxon_pjrt.so")
        mod = types.ModuleType("antenv.axon_hooks")
        mod.get_axon_ntff_profile_hook = lambda: hook
        mod.set_axon_ntff_profile_hook = lambda h: None
        sys.modules["antenv.axon_hooks"] = mod
        antenv.axon_hooks = mod
        return hook is not None
    except Exception:
        return False


def _build(use_qk_bias: bool):
    nc = bacc_mod.Bacc(
        "TRN2",
        target_bir_lowering=False,
        debug=False,
        num_devices=NCORES,
    )

    zm_d = nc.dram_tensor("zm", (CM, N), F32R, kind="ExternalInput").ap()
    zc_d = nc.dram_tensor("zc", (CC, MBLK), F32, kind="ExternalInput").ap()
    gt_d = nc.dram_tensor("gt", (CM, GT_COLS), F32R, kind="ExternalInput").ap()
    wp_d = nc.dram_tensor("wp", (CM, WP_COLS), BF16, kind="ExternalInput").ap()
    if use_qk_bias:
        u_d = nc.dram_tensor("u", (CM, 1), F32R, kind="ExternalInput").ap()
    out_d = nc.dram_tensor("out", (CC, MBLK), F32, kind="ExternalOutput").ap()

    LAG = int(os.environ.get("BASS_PV_LAG", "3"))

    with tile.TileContext(nc) as tc:
        with (
            tc.tile_pool(name="consts", bufs=1) as consts,
            tc.tile_pool(name="epool", bufs=12) as epool,
            tc.tile_pool(name="lpool", bufs=2, space="PSUM") as lpool,
            tc.tile_pool(name="tpool", bufs=1, space="PSUM") as tpool,
            tc.tile_pool(name="spool", bufs=1, space="PSUM") as spool,
            tc.tile_pool(name="opool", bufs=1, space="PSUM") as opool,
        ):
            zm_sb = consts.tile([CM, N], F32R, tag="zm")
            zm_bf = consts.tile([CM, N], BF16, tag="zmbf")
            tq_sb = consts.tile([CM, MBLK], F32R, tag="tq")
            vt_sb = consts.tile([128, N], BF16, tag="vt")  # chunk j at cols 128j
            zc_sb = consts.tile([CC, MBLK], F32, tag="zc")
            gtx_sb = consts.tile([CM, GT_COLS], F32R, tag="gtx")
            wp_sb = consts.tile([CM, WP_COLS], BF16, tag="wp")
            acc0 = consts.tile([128, MBLK], BF16, tag="acc0")
            acc1 = consts.tile([128, MBLK], BF16, tag="acc1")
            accg = consts.tile([128, MBLK], BF16, tag="accg")
            rs_sb = consts.tile([33, 512], F32R, tag="rssb")
            rb_sb = consts.tile([128, MBLK], F32, tag="rbsb")
            tmp_sb = consts.tile([CC, MBLK], F32, tag="tmp")
            out_sb = consts.tile([CC, MBLK], F32, tag="outsb")
            if use_qk_bias:
                u_sb = consts.tile([CM, 1], F32R, tag="u")
                rn_sb = consts.tile([128, NCHUNK], F32, tag="rn")

            wvt_sb = wp_sb[:, WP_WVT]                 # (128, 128) bf16

            # ---- input DMAs fanned across the three DMA-capable
            # sequencers so the issue cost (~0.7us each) doesn't
            # serialize in front of compute ----
            nc.sync.dma_start(zm_sb[:, 0:512], zm_d[:, 0:512])
            nc.scalar.dma_start(gtx_sb[:], gt_d)
            nc.gpsimd.dma_start(zm_sb[:, 512:1024], zm_d[:, 512:1024])
            nc.sync.dma_start(zm_sb[:, 1024:2048], zm_d[:, 1024:2048])
            nc.gpsimd.dma_start(wp_sb[:], wp_d)
            nc.scalar.dma_start(zm_sb[:, 3072:4096], zm_d[:, 3072:4096])
            nc.gpsimd.dma_start(zm_sb[:, 2048:3072], zm_d[:, 2048:3072])
            nc.sync.dma_start(zc_sb[:], zc_d)
            if use_qk_bias:
                nc.gpsimd.dma_start(u_sb[:], u_d)

            out_ps = opool.tile([CC, MBLK], F32, tag="out")
            # PE-side softmax-denominator accumulator: query half h lives
            # on partition 32h (one PSUM bank total).
            s2 = spool.tile([33, 512], F32, tag="s2")

            # tq = (Wq^T Wk) @ zm[:, 0:1024], staged through tpool
            for h in range(2):
                tqp = tpool.tile([128, 512], F32, tag="T")
                nc.tensor.matmul(
                    tqp[:],
                    gtx_sb[:, 0:128],
                    zm_sb[:, h * 512 : (h + 1) * 512],
                    start=True,
                    stop=True,
                )
                nc.vector.tensor_copy(
                    tq_sb[:, h * 512 : (h + 1) * 512], tqp[:]
                )

            def emit_vt_batch(i):
                # vt chunk j = (zm chunk j)^T @ (gamma Wv)^T for j in 4i..4i+3
                nc.vector.tensor_copy(
                    zm_bf[:, i * 512 : (i + 1) * 512],
                    zm_sb[:, i * 512 : (i + 1) * 512].bitcast(F32),
                )
                vps = tpool.tile([128, 512], F32, tag="T")
                for k in range(4):
                    j = 4 * i + k
                    nc.tensor.matmul(
                        vps[:, 128 * k : 128 * (k + 1)],
                        zm_bf[:, 128 * j : 128 * (j + 1)],
                        wvt_sb,
                        start=True,
                        stop=True,
                    )
                nc.vector.tensor_copy(vt_sb[:, i * 512 : (i + 1) * 512], vps[:])
                if use_qk_bias:
                    rnps = tpool.tile([128, 4], F32, tag="T")
                    for k in range(4):
                        j = 4 * i + k
                        nc.tensor.matmul(
                            rnps[:, k : k + 1],
                            zm_sb[:, 128 * j : 128 * (j + 1)],
                            u_sb[:],
                            start=True,
                            stop=True,
                        )
                    nc.vector.tensor_copy(rn_sb[:, 4 * i : 4 * (i + 1)], rnps[:])

            e_tiles = {}

            def emit_s2_add(j, last):
                # chunk j's denominator contribution via two [1,512]
                # ones-matmuls into the s2 accumulator rows
                ej = e_tiles[j] if j in e_tiles else e_done[j]
                for h in range(2):
                    nc.tensor.matmul(
                        s2[32 * h : 32 * h + 1, :],
                        wp_sb[:, WP_ONE],
                        ej[:, 512 * h : 512 * (h + 1)],
                        start=(j == 3),
                        stop=last,
                        skip_group_check=True,
                    )

            e_done = {}

            for j in range(NCHUNK + LAG):
                if j < NCHUNK:
                    if j % 4 == 2 and j // 4 + 1 <= 7:
                        emit_vt_batch(j // 4 + 1)
                    # logits^T chunk j: (keys 128, queries 1024)
                    lps = lpool.tile([128, MBLK], F32, tag="L")
                    for h in range(2):
                        nc.tensor.matmul(
                            lps[:, h * 512 : (h + 1) * 512],
                            zm_sb[:, 128 * j : 128 * (j + 1)],
                            tq_sb[:, h * 512 : (h + 1) * 512],
                            start=True,
                            stop=True,
                        )
                    ej = epool.tile([128, MBLK], BF16, tag="E")
                    bias = rn_sb[:, j : j + 1] if use_qk_bias else 0.0
                    nc.scalar.activation(ej[:], lps[:], AF.Exp, bias=bias)
                    e_tiles[j] = ej
                    if j == 0:
                        emit_vt_batch(0)
                    # softmax denominators, split by measured engine cost:
                    # j%4==3 -> PE (emitted next iteration, lag 1);
                    # j%4==1 -> gpsimd accg; else DVE acc0/acc1.
                    if j % 4 == 1:
                        if j == 1:
                            nc.gpsimd.tensor_copy(accg[:], ej[:])
                        else:
                            nc.gpsimd.tensor_add(accg[:], accg[:], ej[:])
                    elif j % 4 != 3:
                        acc = acc0 if j % 4 == 0 else acc1
                        if j < 3:
                            nc.vector.tensor_copy(acc[:], ej[:])
                        else:
                            nc.vector.tensor_add(acc[:], acc[:], ej[:])
                if j >= 4 and (j - 1) % 4 == 3:
                    emit_s2_add(j - 1, last=False)
                if j >= LAG:
                    jj = j - LAG
                    ej = e_tiles.pop(jj)
                    e_done[jj] = ej
                    for h in range(2):
                        nc.tensor.matmul(
                            out_ps[:, h * 512 : (h + 1) * 512],
                            vt_sb[:, 128 * jj : 128 * (jj + 1)],
                            ej[:, h * 512 : (h + 1) * 512],
                            start=(jj == 0),
                            stop=(jj == NCHUNK - 1),
                        )

            # ---- tail ----
            # fold the DVE/gpsimd accumulators into s2, then 1/s via DVE
            # reciprocal, broadcast across partitions with a K=1 matmul,
            # multiply on DVE, and add zc (+gamma*bv, host-folded) on
            # gpsimd while the quarters stream out.
            for i, src in enumerate([acc0, acc1, accg]):
                for h in range(2):
                    nc.tensor.matmul(
                        s2[32 * h : 32 * h + 1, :],
                        wp_sb[:, WP_ONE],
                        src[:, 512 * h : 512 * (h + 1)],
                        start=False,
                        stop=(i == 2),
                        skip_group_check=True,
                    )
            with nc.allow_low_precision(
                reason="1/s broadcast as f32r for the full-rate PE matmul"
            ):
                nc.vector.reciprocal(rs_sb[:], s2[0:33, :])
            rb = lpool.tile([128, MBLK], F32, tag="L")
            for h in range(2):
                nc.tensor.matmul(
                    rb[:, 512 * h : 512 * (h + 1)],
                    gtx_sb[32 * h : 32 * h + 1, 128:256],
                    rs_sb[32 * h : 32 * h + 1, :],
                    start=True,
                    stop=True,
                )
                if h == 0:
                    nc.scalar.copy(
                        rb_sb[:, 0:512],
                        rb[:, 0:512],
                    )
                else:
                    nc.vector.tensor_copy(
                        rb_sb[:, 512:1024],
                        rb[:, 512:1024],
                    )
            seqs = [nc.sync, nc.scalar, nc.gpsimd, nc.sync]
            for q in range(4):
                sl = slice(q * 256, (q + 1) * 256)
                nc.vector.tensor_tensor(
                    tmp_sb[:, sl], out_ps[:, sl], rb_sb[:, sl], op=ALU.mult
                )
                nc.gpsimd.tensor_add(out_sb[:, sl], tmp_sb[:, sl], zc_sb[:, sl])
                seqs[q].dma_start(out_d[:, sl], out_sb[:, sl])

    nc.compile()
    return nc


_CACHE = {}


def _get_program(use_qk_bias: bool):
    if use_qk_bias not in _CACHE:
        _CACHE[use_qk_bias] = _build(use_qk_bias)
    return _CACHE[use_qk_bias]


def kernel(zc, zm, Wq, bq, Wk, bk, Wv, bv, gamma):
    global LAST_RESULTS
    zc = np.ascontiguousarray(zc, dtype=np.float32)
    zm = np.ascontiguousarray(zm, dtype=np.float32)
    zmf = zm.reshape(B, CM, N)
    zcf = zc.reshape(B, CC, N)

    Wq = np.asarray(Wq, dtype=np.float32)
    Wk = np.asarray(Wk, dtype=np.float32)
    Wv = np.asarray(Wv, dtype=np.float32)
    # tq = gtT^T @ zm must equal (Wk^T Wq)^T... logits^T[n,m] =
    # zm_n^T Wk^T Wq zm_m = zm_n^T tq_m with tq = (Wk^T Wq) zm, and the
    # PE computes tq = lhsT^T @ zm, so lhsT = (Wk^T Wq)^T = Wq^T Wk.
    gtT = (Wq.astype(np.float64).T @ Wk.astype(np.float64)).astype(np.float32)
    gtx = np.concatenate([gtT, np.ones((CM, CM), dtype=np.float32)], axis=1)
    gamma_v = np.float32(np.asarray(gamma).reshape(-1)[0])
    wvt = np.ascontiguousarray((gamma_v * Wv).T).astype(ml_dtypes.bfloat16)
    # gamma*bv folds into the zc addend on the host
    zcb = zcf + (gamma_v * np.asarray(bv, dtype=np.float32))[None, :, None]

    wp = np.concatenate(
        [
            wvt.view(np.uint16),
            np.ones((CM, 2), dtype=ml_dtypes.bfloat16).view(np.uint16),
        ],
        axis=1,
    ).view(ml_dtypes.bfloat16)
    assert wp.shape == (CM, WP_COLS), wp.shape

    use_qk_bias = bool(np.any(bq)) or bool(np.any(bk))
    nc = _get_program(use_qk_bias)

    in_maps = []
    for c in range(NCORES):
        b, jblk = divmod(c, 4)
        m = {
            "zm": np.ascontiguousarray(np.roll(zmf[b], -MBLK * jblk, axis=1)),
            "zc": np.ascontiguousarray(zcb[b][:, MBLK * jblk : MBLK * (jblk + 1)]),
            "gt": gtx,
            "wp": wp,
        }
        if use_qk_bias:
            m["u"] = np.ascontiguousarray(
                (Wk.T @ np.asarray(bq, dtype=np.float32)).reshape(CM, 1)
            )
        in_maps.append(m)

    trace = bool(int(os.environ.get("BASS_KERNEL_TRACE", "0")))
    if trace and not _ensure_ntff_hook():
        trace = False
    res = run_bass_kernel_spmd(
        nc,
        in_maps,
        core_ids=list(range(NCORES)),
        trace=trace,
    )
    LAST_RESULTS = res

    out = np.empty((B, CC, N), dtype=np.float32)
    for c in range(NCORES):
        b, jblk = divmod(c, 4)
        out[b][:, MBLK * jblk : MBLK * (jblk + 1)] = res.results[c]["out"]
    return out.reshape(zc.shape)
